# revision 1
# baseline (speedup 1.0000x reference)
"""DenseCRF mean-field (2,21,80,80) on 8 trn2 NeuronCores.

Math: msg = Q @ (3*Ks + 5*Kb) per batch, Q <- sigmoid(pred - msg), 5 iters.
 - Kb[n,m] = exp(-|f_n-f_m|^2/50) = d_n d_m exp(f_n.f_m/25), f in [0,1]^3,
   so exp(f_n.f_m/25) is Taylor-expanded exactly enough (order 4, rank 35
   monomial feature map; truncation err <= 9e-6 relative).
 - Ks = Ky kron Kx (separable Gaussian), applied exactly as two 80x80
   contractions.
 - Classes never mix => 42 (batch,class) rows split over 8 cores, no
   collectives. Each core: 6 class slots of one batch.

Per-core layouts (P = partition dim):
  canonical state alternates:  Y-layout [80(y), x*8+c]  /  X-layout [80(x), y*8+c]
  phiY [80(y), x*35+r], phiX [80(x), y*35+r]: monomial_r(f) * d  (raw, no coef)
  phiM [35(r), y*80+x]
Iteration (y-type; x-type mirrors with x<->y):
  t[r,c]    = sum_n phi_r(n) Q[n,c]          (80 chunk matmuls, psum acc)
  tS        = w_r * t                         (w_r = -5 * taylor coef^2)
  A[y',xc]  = sum_y (-sqrt3 Ky)[y,y'] Q[y,xc] (2 matmuls)
  AX[x,y'c] = transpose_c(A)                  (6 PE transposes)
  F[x',y'c] = sum_x (sqrt3 Kx)[x,x'] AX       (2 matmuls, start)
  F        += I80.T @ predX                   (2 matmuls)  [adds predictions]
  F[x',y'c]+= sum_r phiM[r, (y',x')] tS[r,c]  (80 matmuls)
  Qnext     = sigmoid(F)                      (ACT, psum->sbuf)
"""

import math

import numpy as np

B, C, H, W = 2, 21, 80, 80
N = H * W
ORDER = 4
GAMMA = 1.0 / 25.0
CW = 6    # class slots per core
CS = 8    # padded class stride
FD = H * CS  # 640, free dim of canonical state
NCORES = 8
NUM_ITERATIONS = 5
BUND = 2001
F32 = np.float32


def _feature_plan():
    """Monomial features of (f0,f1,f2) up to degree ORDER, in the canonical
    order. Returns (parents, weights): parents[r] = (parent_idx, channel) for
    r >= 1; weights[r] = -5 * gamma^k * multinom / k! for feature r."""
    idxs = [()]
    by_ix = {(): 0}
    cur = [()]
    for k in range(1, ORDER + 1):
        new = []
        for ix in cur:
            start = ix[-1] if ix else 0
            for ch in range(start, 3):
                nix = ix + (ch,)
                by_ix[nix] = len(idxs)
                idxs.append(nix)
                new.append(nix)
        cur = new
    parents = []
    weights = []
    for r, ix in enumerate(idxs):
        k = len(ix)
        if r > 0:
            parents.append((by_ix[ix[:-1]], ix[-1]))
        multinom = math.factorial(k)
        for ch in range(3):
            multinom //= math.factorial(ix.count(ch))
        weights.append(-5.0 * GAMMA**k * multinom / math.factorial(k))
    return parents, np.array(weights, dtype=F32)


_PARENTS, _WEIGHTS = _feature_plan()
R = len(_WEIGHTS)  # 35

_CLS_START = [0, 6, 12, 18]
_CLS_WIDTH = [6, 6, 6, 3]


def _spatial_1d(n):
    r = np.arange(n, dtype=np.float64)
    return np.exp(-((r[:, None] - r[None, :]) ** 2) / 18.0)


def _build_in_maps(predictions, image):
    predictions = np.asarray(predictions, dtype=F32)
    image = np.asarray(image, dtype=F32)
    ky = (-math.sqrt(3.0) * _spatial_1d(H)).astype(F32)
    kx = (+math.sqrt(3.0) * _spatial_1d(W)).astype(F32)
    i80 = np.eye(80, dtype=F32)
    wcoef = _WEIGHTS.reshape(R, 1)
    in_maps = []
    for core in range(NCORES):
        b, g = divmod(core, 4)
        cls = (np.arange(CW) + _CLS_START[g]).clip(max=C - 1)
        psel = predictions[b, cls]               # [CW, H, W]
        pY = np.zeros((H, W, CS), dtype=F32)
        pY[:, :, :CW] = psel.transpose(1, 2, 0)  # [y, x, c]
        pX = np.zeros((W, H, CS), dtype=F32)
        pX[:, :, :CW] = psel.transpose(2, 1, 0)  # [x, y, c]
        imY = np.ascontiguousarray(image[b].transpose(1, 2, 0))  # [y, x, ch]
        imX = np.ascontiguousarray(image[b].transpose(2, 1, 0))  # [x, y, ch]
        bund = np.zeros((80, BUND), dtype=F32)
        bund[:, 0:640] = pY.reshape(H, FD)
        bund[:, 640:1280] = pX.reshape(W, FD)
        bund[:, 1280:1520] = imY.reshape(H, 3 * W)
        bund[:, 1520:1760] = imX.reshape(W, 3 * H)
        bund[:, 1760:1840] = ky
        bund[:, 1840:1920] = kx
        bund[:, 1920:2000] = i80
        bund[0:R, 2000] = wcoef[:, 0]
        in_maps.append({"bundle": bund})
    return in_maps


def _assemble(results):
    out = np.zeros((B, C, H, W), dtype=F32)
    for core in range(NCORES):
        b, g = divmod(core, 4)
        w = _CLS_WIDTH[g]
        q = results[core]["qout"].reshape(W, H, CS)  # [x, y, c]
        out[b, _CLS_START[g]:_CLS_START[g] + w] = q[:, :, :w].transpose(2, 1, 0)
    return out


_SKIP = set()


def _build_bass(n_iters=NUM_ITERATIONS, debug_dumps=False):
    import concourse.bass as bass  # noqa: F401
    import concourse.mybir as mybir
    import concourse.tile as tile
    from concourse import bacc

    dt = mybir.dt
    AF = mybir.ActivationFunctionType

    nc = bacc.Bacc("TRN2", target_bir_lowering=False, debug=False)

    # single bundled input: [80, 2001] = predY 640 | predX 640 | imgY 240 |
    # imgX 240 | ky 80 | kx 80 | i80 80 | wcoef col 1 (rows 0:35)
    bund_d = nc.dram_tensor("bundle", [80, BUND], dt.float32,
                            kind="ExternalInput")
    qout_d = nc.dram_tensor("qout", [W, FD], dt.float32, kind="ExternalOutput")
    dumps = {}

    with tile.TileContext(nc) as tc:
        with (
            tc.tile_pool(name="const", bufs=1) as constp,
            tc.tile_pool(name="state", bufs=1) as statep,
            tc.tile_pool(name="work", bufs=2) as workp,
            tc.tile_pool(name="pt", bufs=2, space="PSUM") as ptp,
            tc.tile_pool(name="pbig", bufs=1, space="PSUM") as pbigp,
            tc.tile_pool(name="pfin", bufs=1, space="PSUM") as pfinp,
            tc.tile_pool(name="ptr", bufs=2, space="PSUM") as ptrp,
        ):
            # ---- load inputs: one DMA, tiles are views into the bundle ------
            bund = constp.tile([80, BUND], dt.float32, tag="bundle")
            nc.sync.dma_start(bund[:, 1280:BUND], bund_d[:][:, 1280:BUND])
            nc.sync.dma_start(bund[:, 0:1280], bund_d[:][:, 0:1280])
            b = bund[:]
            predY = b[:, 0:640]
            predX = b[:, 640:1280]
            imgY = b[:, 1280:1520]
            imgX = b[:, 1520:1760]
            kyT = b[:, 1760:1840]
            kxT = b[:, 1840:1920]
            i80 = b[:, 1920:2000]
            wco = b[0:R, 2000:2001]

            # ---- bf16 casts (state/feature path runs bf16; predictions
            # ---- stay f32 where they enter the logits)
            kyb = constp.tile([80, 80], dt.bfloat16, tag="kyb")
            kxb = constp.tile([80, 80], dt.bfloat16, tag="kxb")
            i80b = constp.tile([80, 80], dt.bfloat16, tag="i80b")
            pYb = constp.tile([H, FD], dt.bfloat16, tag="pYb")
            nc.vector.tensor_copy(kyb[:], kyT)
            nc.vector.tensor_copy(kxb[:], kxT)
            nc.vector.tensor_copy(i80b[:], i80)
            nc.scalar.copy(pYb[:], predY)

            # ---- build phi layouts ------------------------------------------
            phiY = constp.tile([H, W * R], dt.float32, tag="phiY")
            phiX = constp.tile([W, H * R], dt.float32, tag="phiX")
            phiM = constp.tile([R, N], dt.float32, tag="phiM")

            def chview(img, ch):       # [80, 80] strided channel view
                return img.rearrange("p (x c) -> p x c", c=3)[:, :, ch]

            def fview(phi, r):         # [80, 80] strided feature view
                return phi[:].rearrange("p (x r) -> p x r", r=R)[:, :, r]

            # group features by parent: one DVE op per parent computes all its
            # children (parent view broadcast with step-0 over the child
            # channels; img channels ch..2 are contiguous).
            by_parent = {}
            for r, (pr, ch) in enumerate(_PARENTS, start=1):
                by_parent.setdefault(pr, []).append((r, ch))

            for img, phi, eng in [(imgX, phiX, nc.vector),
                                  (imgY, phiY, nc.vector)]:
                sq = workp.tile([80, 80], dt.float32, tag="sq")
                m1 = workp.tile([80, 80], dt.float32, tag="m1")
                eng.tensor_mul(sq[:], chview(img, 0), chview(img, 0))
                eng.tensor_mul(m1[:], chview(img, 1), chview(img, 1))
                eng.tensor_add(sq[:], sq[:], m1[:])
                eng.tensor_mul(m1[:], chview(img, 2), chview(img, 2))
                eng.tensor_add(sq[:], sq[:], m1[:])
                # phi[r=0] = d = exp(-sq/50)
                nc.scalar.activation(fview(phi, 0), sq[:], AF.Exp, scale=-0.02)
                img3 = img.rearrange("p (x c) -> p x c", c=3)
                phi3 = phi[:].rearrange("p (x r) -> p x r", r=R)
                for pr, childs in sorted(by_parent.items()):
                    r0, ch0 = childs[0]
                    k = len(childs)
                    # children are consecutive r slots with consecutive chans
                    assert [c for _, c in childs] == list(range(ch0, ch0 + k))
                    assert [r for r, _ in childs] == list(range(r0, r0 + k))
                    par_b = phi3[:, :, pr:pr + 1].broadcast_to([80, 80, k])
                    eng.tensor_mul(phi3[:, :, r0:r0 + k], par_b,
                                   img3[:, :, ch0:ch0 + k])

            # phiM[r, y*80+x] via PE transposes of phiX y-blocks (contiguous
            # dest slices in the y-major layout); batch 6 transposes into one
            # bank-sized psum tile, one copy each, alternating DVE/ACT.
            yb = 0
            g = 0
            while yb < H:
                k = min(6, H - yb)
                tpw = ptrp.tile([R, 480], dt.float32, tag="tr")
                for j in range(k):
                    nc.tensor.transpose(tpw[:, j * 80:(j + 1) * 80],
                                        phiX[:, (yb + j) * R:(yb + j + 1) * R],
                                        i80)
                dst = phiM[:, yb * 80:(yb + k) * 80]
                if g % 2 == 0:
                    nc.vector.tensor_copy(dst, tpw[:, 0:k * 80])
                else:
                    nc.scalar.copy(dst, tpw[:, 0:k * 80])
                yb += k
                g += 1

            # ---- iteration state: f32 canonical + bf16 shadow for the
            # ---- spatial contractions (bf16 Q-state flips borderline rows
            # ---- on ~5% of seeds; spatial-only bf16 is 20/20-seed clean)
            qX = statep.tile([W, FD], dt.float32, tag="qX")
            qY = statep.tile([H, FD], dt.float32, tag="qY")
            qsX = statep.tile([W, FD], dt.bfloat16, tag="qsX")
            qsY = statep.tile([H, FD], dt.bfloat16, tag="qsY")
            qF = statep.tile([80, FD], dt.float32, tag="qF")
            axb = statep.tile([80, FD], dt.bfloat16, tag="axb")
            nc.vector.memset(axb[:], 0.0)

            def iteration(it):
                ytype = (it % 2 == 0)
                if it == 0:
                    qin = predY
                    qin_sp = pYb[:]
                else:
                    qin = qY[:] if ytype else qX[:]
                    qin_sp = qsY[:] if ytype else qsX[:]
                qnext = qF if it == n_iters - 1 else (qX if ytype else qY)
                qs_next = qsX if ytype else qsY
                phiIn = phiY if ytype else phiX
                padd = predX if ytype else predY
                kFirst = kyb[:] if ytype else kxb[:]
                kSecond = kxb[:] if ytype else kyb[:]

                # open the psum group with the prediction add (depends on
                # nothing from this iteration) so it's off the critical path
                pf = pfinp.tile([80, FD], dt.float32, tag="pf")
                nc.tensor.matmul(pf[:, 0:512], i80, padd[:, 0:512],
                                 start=True, stop=False, skip_group_check=True)
                nc.tensor.matmul(pf[:, 512:FD], i80, padd[:, 512:FD],
                                 start=True, stop=False, skip_group_check=True)

                # spatial first contraction
                pa = pbigp.tile([80, FD], dt.float32, tag="pa")
                nc.tensor.matmul(pa[:, 0:512], kFirst, qin_sp[:, 0:512],
                                 start=True, stop=True)
                nc.tensor.matmul(pa[:, 512:FD], kFirst, qin_sp[:, 512:FD],
                                 start=True, stop=True)
                asb = workp.tile([80, FD], dt.bfloat16, tag="asb")
                nc.scalar.copy(asb[:, 0:320], pa[:, 0:320])
                nc.vector.tensor_copy(asb[:, 320:FD], pa[:, 320:FD])

                # t[r,c] accumulation over 80 blocks (fills PE while the
                # psum->sbuf copies run), then the bilateral accumulation
                pt = ptp.tile([R, CW], dt.float32, tag="pt")
                for j in (range(80) if "mm1" not in _SKIP else range(1)):
                    nc.tensor.matmul(
                        pt[:], phiIn[:, j * R:(j + 1) * R],
                        qin[:, j * CS:j * CS + CW],
                        start=(j == 0), stop=(j == 79))
                tS = workp.tile([R, CW], dt.float32, tag="tS")
                nc.vector.tensor_scalar_mul(tS[:], pt[:], wco)
                phiM3 = phiM[:].rearrange("p (y x) -> p y x", x=W)
                for j in (range(80) if "mm2" not in _SKIP else range(1)):
                    nc.tensor.matmul(pf[:, j * CS:j * CS + CW],
                                     (phiM[:, j * 80:(j + 1) * 80] if ytype
                                      else phiM3[:, :, j]), tS[:],
                                     start=False, stop=False,
                                     skip_group_check=True)

                # per-class transpose of A
                tpc = ptrp.tile([80, 480], dt.bfloat16, tag="tr")
                for c in (range(CW) if "ct" not in _SKIP else range(1)):
                    srcv = asb[:].rearrange("p (b s) -> p b s", s=CS)[:, :, c]
                    nc.tensor.transpose(tpc[:, c * 80:(c + 1) * 80], srcv,
                                        i80b[:])
                tin = tpc[:].rearrange("p (c b) -> p c b", c=CW)
                tout = axb[:].rearrange("p (b s) -> p s b", s=CS)[:, 0:CW, :]
                nc.vector.tensor_copy(tout, tin)

                # spatial second contraction closes the accumulation
                nc.tensor.matmul(pf[:, 0:512], kSecond, axb[:, 0:512],
                                 start=False, stop=True, skip_group_check=True)
                nc.tensor.matmul(pf[:, 512:FD], kSecond, axb[:, 512:FD],
                                 start=False, stop=True, skip_group_check=True)

                # bf16 shadow sigmoid first: unblocks the next iteration's
                # spatial matmul before the f32 sigmoids finish. Identical
                # numerics to casting the f32 result (ACT rounds its internal
                # f32 sigmoid to bf16).
                if it < n_iters - 1:
                    nc.scalar.activation(qs_next[:, 0:512], pf[:, 0:512],
                                         AF.Sigmoid)
                    nc.scalar.activation(qs_next[:, 512:FD], pf[:, 512:FD],
                                         AF.Sigmoid)
                nc.scalar.activation(qnext[:, 0:512], pf[:, 0:512], AF.Sigmoid)
                nc.scalar.activation(qnext[:, 512:FD], pf[:, 512:FD], AF.Sigmoid)
                return qnext

            qfin = None
            for it in range(n_iters):
                qfin = iteration(it)

            nc.sync.dma_start(qout_d[:][:, 0:512], qfin[:, 0:512])
            nc.sync.dma_start(qout_d[:][:, 512:FD], qfin[:, 512:FD])
            if debug_dumps:
                for nm, t_ in [("phiM", phiM), ("phiY", phiY)]:
                    d = nc.dram_tensor("dump_" + nm, list(t_[:].shape),
                                       dt.float32, kind="ExternalOutput")
                    nc.sync.dma_start(d[:], t_[:])
                    dumps[nm] = d

    nc.compile()
    return nc


def kernel(predictions, image):
    from concourse.bass_utils import run_bass_kernel_spmd

    nc = _build_bass()
    in_maps = _build_in_maps(predictions, image)
    last_err = None
    for _attempt in range(3):
        try:
            res = run_bass_kernel_spmd(nc, in_maps, core_ids=list(range(NCORES)))
            return _assemble(res.results)
        except Exception as e:  # transient device wedges happen; retry
            last_err = e
    raise last_err

